# revision 35
# baseline (speedup 1.0000x reference)
"""Block-sparse attention Trainium2 kernel (v3, bf16 transposed-AV).

Reference: nn.MultiheadAttention-style block-sparse attention, B=1, L=4096,
D=1024, H=16, head_dim=64, block=128, global blocks {0, 24}.

Sharding: head-parallel across 8 cores (2 heads/core); host sums the 8
partial out-projections. The whole dataflow is bf16 (inputs pre-cast on
host): matmul operands bf16 into f32 PSUM, exps emit bf16, the partial
output is written bf16 and summed in f32 on host. Attention-value products
are computed in transposed form (outT = v_aug.T @ expT); softmax
denominators ride along as an extra ones-column of the augmented V;
normalization is a reciprocal + PE outer-product broadcast + one
elementwise multiply per 512-wide chunk.
"""

import sys

sys.path.insert(0, "/opt/trn_rl_repo")
import numpy as np

D = 1024
L = 4096
H = 16
HD = 64
NB = 32
GLOB = (0, 24)
P = 128
SCALE = 1.0 / 8.0

PHASES = "full"  # dev knob: "qkv" / "attn1" time sub-phases via mb_phase.py

_CACHE = {}


def _build_nc(reps=1):
    import contextlib

    import concourse.mybir as mybir
    import concourse.tile as tile
    from concourse import bacc
    from concourse.masks import make_identity

    f32 = mybir.dt.float32
    f32r = mybir.dt.float32r
    bf16 = mybir.dt.bfloat16
    Act = mybir.ActivationFunctionType
    AluMult = mybir.AluOpType.mult

    nc = bacc.Bacc("TRN2", target_bir_lowering=False, debug=False, num_devices=8)
    xT = nc.dram_tensor("xT", [D, L], bf16, kind="ExternalInput")
    wq = nc.dram_tensor("wq", [P, D], bf16, kind="ExternalInput")
    wk = nc.dram_tensor("wk", [P, D], bf16, kind="ExternalInput")
    wv = nc.dram_tensor("wv", [P, D], bf16, kind="ExternalInput")
    wo = nc.dram_tensor("wo", [P, D], bf16, kind="ExternalInput")
    bq = nc.dram_tensor("bq", [P, 1], f32, kind="ExternalInput")
    bk = nc.dram_tensor("bk", [P, 1], f32, kind="ExternalInput")
    out = nc.dram_tensor("out", [L, D], bf16, kind="ExternalOutput")

    with tile.TileContext(nc) as tc:
        with (
            tc.tile_pool(name="const", bufs=1) as constp,
            tc.tile_pool(name="stream", bufs=3) as streamp,
            tc.tile_pool(name="expb", bufs=6) as expp,
            tc.tile_pool(name="small", bufs=4) as smallp,
            tc.tile_pool(name="ps_big", bufs=2, space="PSUM") as ps_big,
            tc.tile_pool(name="ps_med", bufs=3, space="PSUM") as ps_med,
            tc.tile_pool(name="ps_av", bufs=3, space="PSUM") as ps_av,
        ):
            # ---------- constants / persistent buffers
            ident = constp.tile([P, P], bf16, tag="ident")
            make_identity(nc, ident[:])
            ones_col = constp.tile([P, 1], bf16, tag="ones")
            nc.vector.memset(ones_col[:], 1.0)

            wq_r = constp.tile([P, D], bf16, tag="wq_r")
            wk_r = constp.tile([P, D], bf16, tag="wk_r")
            wv_r = constp.tile([P, D], bf16, tag="wv_r")
            wo_r = constp.tile([P, D], bf16, tag="wo_r")
            for dram, tr in ((wq, wq_r), (wk, wk_r), (wv, wv_r), (wo, wo_r)):
                nc.sync.dma_start(tr[:], dram[:])
            bq_t = constp.tile([P, 1], f32, tag="bq")
            bk_t = constp.tile([P, 1], f32, tag="bk")
            nc.sync.dma_start(bq_t[:], bq[:])
            nc.sync.dma_start(bk_t[:], bk[:])

            qT = constp.tile([P, L], bf16, tag="qT")
            kT = constp.tile([P, L], bf16, tag="kT")
            vTf = constp.tile([P, L], bf16, tag="vTf")
            # augmented-V blocks: per block 130 cols = [v.T lo | ones | v.T hi | ones]
            # each head's 65-col window puts data in psum rows 0:64, l in row 64.
            vn = constp.tile([P, NB * 130], bf16, tag="vn")
            qg = constp.tile([P, 256], bf16, tag="qg")
            gout = constp.tile([P, 256], bf16, tag="gout")
            for _b in range(NB):
                nc.vector.tensor_copy(vn[:, _b * 130 + 64:_b * 130 + 65], ones_col[:])
                nc.vector.tensor_copy(vn[:, _b * 130 + 129:_b * 130 + 130], ones_col[:])

            loop_ctx = tc.For_i(0, reps, 1) if reps > 1 else contextlib.nullcontext()
            with loop_ctx:
                _body(nc, tc, mybir, Act, f32, f32r, bf16, AluMult, locals())

    nc.compile()
    return nc


def _body(nc, tc, mybir, Act, f32, f32r, bf16, AluMult, env):
    constp = env["constp"]; streamp = env["streamp"]; expp = env["expp"]; smallp = env["smallp"]
    ps_big = env["ps_big"]; ps_med = env["ps_med"]; ps_av = env["ps_av"]
    ident = env["ident"]; ones_col = env["ones_col"]
    wq_r = env["wq_r"]; wk_r = env["wk_r"]; wv_r = env["wv_r"]; wo_r = env["wo_r"]
    bq_t = env["bq_t"]; bk_t = env["bk_t"]
    qT = env["qT"]; kT = env["kT"]; vTf = env["vTf"]; vn = env["vn"]
    qg = env["qg"]; gout = env["gout"]
    xT = env["xT"]; out = env["out"]
    AluAdd = mybir.AluOpType.add

    # ---------- phase A: qkv projections + fused v-transpose, per quad
    def do_quad(quad):
        xrs = []
        for kt in range(8):
            xraw = streamp.tile([P, 1024], bf16, tag="xraw", bufs=16)
            nc.sync.dma_start(
                xraw[:], xT[kt * P:(kt + 1) * P, quad * 1024:(quad + 1) * 1024]
            )
            xrs.append(xraw)
        for sub in range(2):
            n = quad * 2 + sub
            sl = slice(n * 512, (n + 1) * 512)
            for wt, dest, bias in (
                (wq_r, qT, bq_t),
                (wk_r, kT, bk_t),
                (wv_r, vTf, None),
            ):
                pp = ps_big.tile([P, 512], f32, tag="psbig")
                for kt in range(8):
                    nc.tensor.matmul(
                        pp[:], wt[:, kt * P:(kt + 1) * P],
                        xrs[kt][:, sub * 512:(sub + 1) * 512],
                        start=kt == 0, stop=kt == 7,
                    )
                if bias is not None:
                    nc.scalar.activation(dest[:, sl], pp[:], Act.Identity, bias=bias[:])
                else:
                    nc.vector.tensor_copy(dest[:, sl], pp[:])
        for b in range(8 * quad, 8 * quad + 8):
            pst = ps_av.tile([P, P], bf16, tag="psav", name=f"pst{b}")
            nc.tensor.transpose(pst[:], vTf[:, b * P:(b + 1) * P], ident[:])
            base = b * 130
            nc.vector.tensor_copy(vn[:, base:base + 64], pst[:, 0:64])
            nc.vector.tensor_copy(vn[:, base + 65:base + 129], pst[:, 64:128])

    do_quad(0)
    do_quad(3)
    # stage global-q columns (available after quads 0 and 3)
    nc.vector.tensor_copy(qg[:, 0:128], qT[:, 0:128])
    nc.vector.tensor_copy(qg[:, 128:256], qT[:, GLOB[1] * P:(GLOB[1] + 1) * P])

    def vslice(blk, h):
        return vn[:, blk * 130 + h * 65: blk * 130 + (h + 1) * 65]

    def norm_prep(src, lo, hi, l_in_sbuf=False):
        # src [65, W+]: rows 0:64 = unnormalized outT, row 64 = l.
        # Returns [64, W] broadcast of 1/l. The l-row extraction uses Act
        # Identity (same act table as Exp — no table reload) because any
        # [1, W] DVE op runs single-lane (~2.9us on HW); the reciprocal runs
        # AFTER the broadcast so it is partition-parallel (~0.6us).
        W = hi - lo
        # NOTE: partition_broadcast on HW reads physical partition 0 of its
        # input — an AP at partition offset 64 silently misreads (CoreSim
        # follows the offset). Always extract l into a partition-0 tile.
        t = smallp.tile([1, 512], f32, tag="lsb")
        nc.scalar.activation(t[0:1, 0:W], src[64:65, lo:hi], Act.Identity)
        lsb = t[0:1, 0:W]
        bsb = smallp.tile([64, 512], f32, tag="bsb")
        nc.gpsimd.partition_broadcast(bsb[0:64, 0:W], lsb)
        rsb = smallp.tile([64, 512], f32, tag="rsb")
        nc.vector.reciprocal(rsb[0:64, 0:W], bsb[0:64, 0:W])
        return rsb

    def norm_mult(src, bsb, lo, hi, dest):
        W = hi - lo
        nc.vector.tensor_tensor(
            dest, src[0:64, lo:hi], bsb[0:64, 0:W], AluMult
        )

    def normalize_emit(src, h, lo, hi, dest, l_in_sbuf=False):
        norm_mult(src, norm_prep(src, lo, hi, l_in_sbuf), lo, hi, dest)

    # ---------- global qtiles (0 and 24): attend to all 32 blocks.
    # AV accumulates 8 kblocks per rotating PSUM tile, drained into an SBUF
    # accumulator (no long-lived PSUM bank).
    def do_global():
      for h in (0, 1):
        hs = slice(h * 64, (h + 1) * 64)
        gacc = None
        for grp in range(4):  # 8 kblocks per group
            pgp = ps_av.tile([65, 256], f32, tag="psav", name=f"pgp{h}_{grp}")
            for kb2 in range(4 * grp, 4 * grp + 4):
                psg = ps_med.tile([P, 512], f32, tag="psmed")
                for half in (0, 1):
                    kb = 2 * kb2 + half
                    nc.tensor.matmul(
                        psg[:, half * 256:(half + 1) * 256],
                        kT[hs, kb * P:(kb + 1) * P], qg[hs, :],
                        start=True, stop=True,
                    )
                eg = expp.tile([P, 512], bf16, tag="gexp")
                nc.scalar.activation(eg[:], psg[:], Act.Exp, scale=SCALE)
                for half in (0, 1):
                    kb = 2 * kb2 + half
                    nc.tensor.matmul(
                        pgp[:], vslice(kb, h), eg[:, half * 256:(half + 1) * 256],
                        start=kb == 8 * grp, stop=kb == 8 * grp + 7,
                    )
            nxt = smallp.tile([65, 256], f32, tag="gacc")
            if gacc is None:
                nc.vector.tensor_copy(nxt[:], pgp[:])
            else:
                nc.vector.tensor_tensor(nxt[:], gacc[:], pgp[:], AluAdd)
            gacc = nxt
        normalize_emit(gacc, h, 0, 256, gout[h * 64:(h + 1) * 64, :],
                       l_in_sbuf=True)

    # ---------- chunk pipeline: S(c) scores+exps, A(c) AV+normalize,
    # O(c) out-projection. Emitted as S(c) / A(c-1) / O(c-2) so every
    # cross-engine dependency has ~a full chunk of slack to absorb the
    # ~150ns semaphore handoff latency of the hardware.
    otrs, egss, edss = {}, {}, {}

    def chunk_info(c):
        glob_in_chunk = [g for g in GLOB if g // 4 == c]
        lo = 128 if glob_in_chunk else 0
        qts = [4 * c + i for i in range(4) if (4 * c + i) not in GLOB]
        return glob_in_chunk, lo, qts

    def do_scores(c):
        _, _, qts = chunk_info(c)
        nq = len(qts)
        for h in (0, 1):
            hs = slice(h * 64, (h + 1) * 64)
            for g in GLOB:
                psg = ps_med.tile([P, 512], f32, tag="psmed")
                nc.tensor.matmul(
                    psg[:], kT[hs, g * P:(g + 1) * P],
                    qT[hs, c * 512:(c + 1) * 512],
                    start=True, stop=True,
                )
                eg = expp.tile([P, 512], bf16, tag="exp", bufs=12)
                nc.scalar.activation(eg[:], psg[:], Act.Exp, scale=SCALE)
                egss[c, g, h] = eg
            psd = ps_med.tile([P, 512], f32, tag="psmed")
            for idx, j in enumerate(qts):
                nc.tensor.matmul(
                    psd[:, idx * P:(idx + 1) * P],
                    kT[hs, j * P:(j + 1) * P], qT[hs, j * P:(j + 1) * P],
                    start=True, stop=True, skip_group_check=True,
                )
            ed = expp.tile([P, 512], bf16, tag="exp", bufs=12)
            nc.scalar.activation(
                ed[:, 0:nq * P], psd[:, 0:nq * P], Act.Exp, scale=SCALE
            )
            edss[c, h] = ed

    def do_av(c):
        glob_in_chunk, lo, qts = chunk_info(c)
        otr = smallp.tile([P, 512], bf16, tag="otr", name=f"otr{c}")
        otrs[c] = otr
        if glob_in_chunk:
            g = glob_in_chunk[0]
            gq_col = 0 if g == 0 else 128
            nc.vector.tensor_copy(otr[:, 0:128], gout[:, gq_col:gq_col + 128])
        pcs = {}
        for h in (0, 1):
            pc = ps_av.tile([65, 512], f32, tag="psav")
            pcs[h] = pc
            nc.tensor.matmul(pc[:, lo:512], vslice(GLOB[0], h),
                             egss[c, GLOB[0], h][:, lo:512],
                             start=True, stop=False)
            nc.tensor.matmul(pc[:, lo:512], vslice(GLOB[1], h),
                             egss[c, GLOB[1], h][:, lo:512],
                             start=False, stop=True)
            for idx, j in enumerate(qts):
                off = (j - 4 * c) * P
                nc.tensor.matmul(pc[:, off:off + P], vslice(j, h),
                                 edss[c, h][:, idx * P:(idx + 1) * P],
                                 start=False, stop=True,
                                 skip_group_check=True)  # sub-region accumulate
        # both l-extracts+broadcasts first, then both divides: DVE never
        # sits head-of-line waiting for a Pool broadcast round-trip.
        bsbs = {h: norm_prep(pcs[h], lo, 512) for h in (0, 1)}
        for h in (0, 1):
            norm_mult(pcs[h], bsbs[h], lo, 512, otr[h * 64:(h + 1) * 64, lo:512])

    def do_outproj(c):
        otr = otrs[c]
        for t in range(4):
            j = 4 * c + t
            osb = streamp.tile([P, D], bf16, tag="osb")
            for half in (0, 1):
                pso = ps_big.tile([P, 512], f32, tag="psbig")
                nc.tensor.matmul(
                    pso[:], otr[:, t * P:(t + 1) * P],
                    wo_r[:, half * 512:(half + 1) * 512],
                    start=True, stop=True,
                )
                if half == 0:
                    nc.vector.tensor_copy(osb[:, 0:512], pso[:])
                else:
                    nc.scalar.activation(osb[:, 512:1024], pso[:], Act.Identity)
            nc.gpsimd.dma_start(out[j * P:(j + 1) * P, :], osb[:])

    if PHASES == "qkv":
        do_quad(1)
        do_quad(2)
        return
    if PHASES == "attn1":
        do_quad(1); do_quad(2)
        do_scores(1)
        do_av(1)
        do_outproj(1)
        return
    if PHASES == "attn4":
        do_quad(1); do_quad(2)
        do_scores(1)
        do_scores(7); do_av(1)
        do_scores(2); do_av(7); do_outproj(1)
        do_scores(3); do_av(2); do_outproj(7)
        do_av(3); do_outproj(2)
        do_outproj(3)
        return
    # wavefront: S one chunk ahead of A, two ahead of O; quads 1 and 2 are
    # interleaved into the early slots (chunks 1 and 7 only need quads 0+3,
    # emitted above) so QKV matmuls fill the attention phase's wait gaps.
    # The global block (do_global) sits before the gout-dependent chunks 0/6.
    do_scores(1)
    do_quad(1)
    do_scores(7); do_av(1)
    do_quad(2)
    do_scores(2); do_av(7); do_outproj(1)
    do_scores(3); do_av(2); do_outproj(7)
    do_scores(4); do_av(3); do_outproj(2)
    do_scores(5); do_av(4); do_outproj(3)
    do_global(); do_outproj(4)
    do_av(5); do_outproj(5)
    do_scores(0); do_scores(6)
    do_av(0); do_outproj(0)
    do_av(6); do_outproj(6)


def _get_nc(reps=1):
    key = ("nc", reps)
    if key not in _CACHE:
        _CACHE[key] = _build_nc(reps)
    return _CACHE[key]


def _bf16(a):
    import ml_dtypes

    return np.asarray(a, dtype=np.float32).astype(ml_dtypes.bfloat16)


def _prep_inputs(x, w_qkv, b_qkv):
    x2 = np.asarray(x, dtype=np.float32).reshape(L, D)
    xT = _bf16(np.ascontiguousarray(x2.T))
    w_qkv = np.asarray(w_qkv, dtype=np.float32)
    b_qkv = np.asarray(b_qkv, dtype=np.float32)

    def tile_w(w_slice):
        wt = w_slice.T
        return _bf16(np.ascontiguousarray(
            wt.reshape(8, P, P).transpose(1, 0, 2).reshape(P, D)
        ))

    maps = []
    for c in range(8):
        a = 2 * c * HD
        b = a + 2 * HD
        maps.append({
            "xT": xT,
            "wq": tile_w(w_qkv[a:b, :]),
            "wk": tile_w(w_qkv[D + a:D + b, :]),
            "wv": tile_w(w_qkv[2 * D + a:2 * D + b, :]),
            "bq": np.ascontiguousarray(b_qkv[a:b].reshape(P, 1)),
            "bk": np.ascontiguousarray(b_qkv[D + a:D + b].reshape(P, 1)),
        })
    return maps


def kernel(x, w_qkv, b_qkv, w_out, b_out):
    from concourse.bass_utils import run_bass_kernel_spmd

    x = np.asarray(x, dtype=np.float32)
    w_qkv = np.asarray(w_qkv, dtype=np.float32)
    b_qkv = np.asarray(b_qkv, dtype=np.float32)
    w_out = np.asarray(w_out, dtype=np.float32)
    b_out = np.asarray(b_out, dtype=np.float32)

    nc = _get_nc()
    maps = _prep_inputs(x, w_qkv, b_qkv)
    for c in range(8):
        a = 2 * c * HD
        b = a + 2 * HD
        maps[c]["wo"] = _bf16(np.ascontiguousarray(w_out[:, a:b].T))

    res = run_bass_kernel_spmd(nc, maps, core_ids=list(range(8)))

    total = res.results[0]["out"].astype(np.float32)
    for c in range(1, 8):
        total += res.results[c]["out"].astype(np.float32)
    const_row = b_qkv[2 * D:3 * D] @ w_out.T + b_out
    total += const_row[None, :]
    return total.reshape(x.shape).astype(np.float32)


# revision 37
# speedup vs baseline: 1.4338x; 1.4338x over previous
"""Block-sparse attention Trainium2 kernel (v3, bf16 transposed-AV).

Reference: nn.MultiheadAttention-style block-sparse attention, B=1, L=4096,
D=1024, H=16, head_dim=64, block=128, global blocks {0, 24}.

Sharding: head-parallel across 8 cores (2 heads/core); host sums the 8
partial out-projections. The whole dataflow is bf16 (inputs pre-cast on
host): matmul operands bf16 into f32 PSUM, exps emit bf16, the partial
output is written bf16 and summed in f32 on host. Attention-value products
are computed in transposed form (outT = v_aug.T @ expT); softmax
denominators ride along as an extra ones-column of the augmented V;
normalization is a reciprocal + PE outer-product broadcast + one
elementwise multiply per 512-wide chunk.
"""

import sys

sys.path.insert(0, "/opt/trn_rl_repo")
import numpy as np

D = 1024
L = 4096
H = 16
HD = 64
NB = 32
GLOB = (0, 24)
P = 128
SCALE = 1.0 / 8.0

PHASES = "full"  # dev knob: "qkv" / "attn1" time sub-phases via mb_phase.py

_CACHE = {}


def _build_nc(reps=1):
    import contextlib

    import concourse.mybir as mybir
    import concourse.tile as tile
    from concourse import bacc
    from concourse.masks import make_identity

    f32 = mybir.dt.float32
    f32r = mybir.dt.float32r
    bf16 = mybir.dt.bfloat16
    Act = mybir.ActivationFunctionType
    AluMult = mybir.AluOpType.mult

    nc = bacc.Bacc("TRN2", target_bir_lowering=False, debug=False, num_devices=8)
    xT = nc.dram_tensor("xT", [D, L], bf16, kind="ExternalInput")
    wq = nc.dram_tensor("wq", [P, D], bf16, kind="ExternalInput")
    wk = nc.dram_tensor("wk", [P, D], bf16, kind="ExternalInput")
    wv = nc.dram_tensor("wv", [P, D], bf16, kind="ExternalInput")
    wo = nc.dram_tensor("wo", [P, D], bf16, kind="ExternalInput")
    bq = nc.dram_tensor("bq", [P, 1], f32, kind="ExternalInput")
    bk = nc.dram_tensor("bk", [P, 1], f32, kind="ExternalInput")
    out = nc.dram_tensor("out", [L, D], bf16, kind="ExternalOutput")

    with tile.TileContext(nc) as tc:
        with (
            tc.tile_pool(name="const", bufs=1) as constp,
            tc.tile_pool(name="stream", bufs=3) as streamp,
            tc.tile_pool(name="expb", bufs=6) as expp,
            tc.tile_pool(name="small", bufs=4) as smallp,
            tc.tile_pool(name="ps_big", bufs=2, space="PSUM") as ps_big,
            tc.tile_pool(name="ps_med", bufs=3, space="PSUM") as ps_med,
            tc.tile_pool(name="ps_av", bufs=3, space="PSUM") as ps_av,
        ):
            # ---------- constants / persistent buffers
            ident = constp.tile([P, P], bf16, tag="ident")
            make_identity(nc, ident[:])
            ones_col = constp.tile([P, 1], bf16, tag="ones")
            nc.vector.memset(ones_col[:], 1.0)

            wq_r = constp.tile([P, D], bf16, tag="wq_r")
            wk_r = constp.tile([P, D], bf16, tag="wk_r")
            wv_r = constp.tile([P, D], bf16, tag="wv_r")
            wo_r = constp.tile([P, D], bf16, tag="wo_r")
            for dram, tr in ((wq, wq_r), (wk, wk_r), (wv, wv_r), (wo, wo_r)):
                nc.sync.dma_start(tr[:], dram[:])
            bq_t = constp.tile([P, 1], f32, tag="bq")
            bk_t = constp.tile([P, 1], f32, tag="bk")
            nc.sync.dma_start(bq_t[:], bq[:])
            nc.sync.dma_start(bk_t[:], bk[:])

            qT = constp.tile([P, L], bf16, tag="qT")
            kT = constp.tile([P, L], bf16, tag="kT")
            vTf = constp.tile([P, L], bf16, tag="vTf")
            # augmented-V blocks: per block 130 cols = [v.T lo | ones | v.T hi | ones]
            # each head's 65-col window puts data in psum rows 0:64, l in row 64.
            vn = constp.tile([P, NB * 130], bf16, tag="vn")
            qg = constp.tile([P, 256], bf16, tag="qg")
            gout = constp.tile([P, 256], bf16, tag="gout")
            for _b in range(NB):
                nc.vector.tensor_copy(vn[:, _b * 130 + 64:_b * 130 + 65], ones_col[:])
                nc.vector.tensor_copy(vn[:, _b * 130 + 129:_b * 130 + 130], ones_col[:])

            loop_ctx = tc.For_i(0, reps, 1) if reps > 1 else contextlib.nullcontext()
            with loop_ctx:
                _body(nc, tc, mybir, Act, f32, f32r, bf16, AluMult, locals())

    nc.compile()
    return nc


def _body(nc, tc, mybir, Act, f32, f32r, bf16, AluMult, env):
    constp = env["constp"]; streamp = env["streamp"]; expp = env["expp"]; smallp = env["smallp"]
    ps_big = env["ps_big"]; ps_med = env["ps_med"]; ps_av = env["ps_av"]
    ident = env["ident"]; ones_col = env["ones_col"]
    wq_r = env["wq_r"]; wk_r = env["wk_r"]; wv_r = env["wv_r"]; wo_r = env["wo_r"]
    bq_t = env["bq_t"]; bk_t = env["bk_t"]
    qT = env["qT"]; kT = env["kT"]; vTf = env["vTf"]; vn = env["vn"]
    qg = env["qg"]; gout = env["gout"]
    xT = env["xT"]; out = env["out"]
    AluAdd = mybir.AluOpType.add

    # ---------- phase A: qkv projections + fused v-transpose, per quad
    def do_quad(quad):
        xrs = []
        for kt in range(8):
            xraw = streamp.tile([P, 1024], bf16, tag="xraw", bufs=16)
            nc.sync.dma_start(
                xraw[:], xT[kt * P:(kt + 1) * P, quad * 1024:(quad + 1) * 1024]
            )
            xrs.append(xraw)
        for sub in range(2):
            n = quad * 2 + sub
            sl = slice(n * 512, (n + 1) * 512)
            for wt, dest, bias in (
                (wq_r, qT, bq_t),
                (wk_r, kT, bk_t),
                (wv_r, vTf, None),
            ):
                pp = ps_big.tile([P, 512], f32, tag="psbig")
                for kt in range(8):
                    nc.tensor.matmul(
                        pp[:], wt[:, kt * P:(kt + 1) * P],
                        xrs[kt][:, sub * 512:(sub + 1) * 512],
                        start=kt == 0, stop=kt == 7,
                    )
                if bias is not None:
                    nc.scalar.activation(dest[:, sl], pp[:], Act.Identity, bias=bias[:])
                else:
                    nc.vector.tensor_copy(dest[:, sl], pp[:])
        for b in range(8 * quad, 8 * quad + 8):
            pst = ps_av.tile([P, P], bf16, tag="psav", name=f"pst{b}")
            nc.tensor.transpose(pst[:], vTf[:, b * P:(b + 1) * P], ident[:])
            base = b * 130
            nc.vector.tensor_copy(vn[:, base:base + 64], pst[:, 0:64])
            nc.vector.tensor_copy(vn[:, base + 65:base + 129], pst[:, 64:128])

    do_quad(0)
    do_quad(3)
    # stage global-q columns (available after quads 0 and 3)
    nc.vector.tensor_copy(qg[:, 0:128], qT[:, 0:128])
    nc.vector.tensor_copy(qg[:, 128:256], qT[:, GLOB[1] * P:(GLOB[1] + 1) * P])

    def vslice(blk, h):
        return vn[:, blk * 130 + h * 65: blk * 130 + (h + 1) * 65]

    def norm_prep(src, lo, hi, l_in_sbuf=False):
        # src [65, W+]: rows 0:64 = unnormalized outT, row 64 = l.
        # Returns [64, W] broadcast of 1/l. reciprocal_approx_fast: the exact
        # DVE reciprocal costs ~6ns per free-element on HW (~2.9us per row),
        # the approx one ~0.7ns; softmax denominators don't need the bits.
        # NOTE: partition_broadcast on HW reads physical partition 0 of its
        # input — an AP at partition offset 64 silently misreads (CoreSim
        # follows the offset) — so linv lands in a partition-0 tile first.
        W = hi - lo
        # approx_fast (like partition_broadcast) misreads partition-offset
        # APs on HW, so the l row is first moved to a partition-0 tile by an
        # Act Identity (same act table as Exp — no table reload).
        lsb = smallp.tile([1, 512], f32, tag="lsb")
        nc.scalar.activation(lsb[0:1, 0:W], src[64:65, lo:hi], Act.Identity)
        linv = smallp.tile([1, 512], f32, tag="linv")
        with nc.allow_low_precision(reason="softmax denom tolerates approx"):
            nc.vector.reciprocal_approx_fast(linv[0:1, 0:W], lsb[0:1, 0:W])
        bsb = smallp.tile([64, 512], f32, tag="bsb")
        nc.gpsimd.partition_broadcast(bsb[0:64, 0:W], linv[0:1, 0:W])
        return bsb

    def norm_mult(src, bsb, lo, hi, dest):
        W = hi - lo
        nc.vector.tensor_tensor(
            dest, src[0:64, lo:hi], bsb[0:64, 0:W], AluMult
        )

    def normalize_emit(src, h, lo, hi, dest, l_in_sbuf=False):
        norm_mult(src, norm_prep(src, lo, hi, l_in_sbuf), lo, hi, dest)

    # ---------- global qtiles (0 and 24): attend to all 32 blocks.
    # AV accumulates 8 kblocks per rotating PSUM tile, drained into an SBUF
    # accumulator (no long-lived PSUM bank).
    def do_global():
      for h in (0, 1):
        hs = slice(h * 64, (h + 1) * 64)
        gacc = None
        for grp in range(4):  # 8 kblocks per group
            pgp = ps_av.tile([65, 256], f32, tag="psav", name=f"pgp{h}_{grp}")
            for kb2 in range(4 * grp, 4 * grp + 4):
                psg = ps_med.tile([P, 512], f32, tag="psmed")
                for half in (0, 1):
                    kb = 2 * kb2 + half
                    nc.tensor.matmul(
                        psg[:, half * 256:(half + 1) * 256],
                        kT[hs, kb * P:(kb + 1) * P], qg[hs, :],
                        start=True, stop=True,
                    )
                eg = expp.tile([P, 512], bf16, tag="gexp")
                nc.scalar.activation(eg[:], psg[:], Act.Exp, scale=SCALE)
                for half in (0, 1):
                    kb = 2 * kb2 + half
                    nc.tensor.matmul(
                        pgp[:], vslice(kb, h), eg[:, half * 256:(half + 1) * 256],
                        start=kb == 8 * grp, stop=kb == 8 * grp + 7,
                    )
            nxt = smallp.tile([65, 256], f32, tag="gacc")
            if gacc is None:
                nc.vector.tensor_copy(nxt[:], pgp[:])
            else:
                nc.vector.tensor_tensor(nxt[:], gacc[:], pgp[:], AluAdd)
            gacc = nxt
        normalize_emit(gacc, h, 0, 256, gout[h * 64:(h + 1) * 64, :],
                       l_in_sbuf=True)

    # ---------- chunk pipeline: S(c) scores+exps, A(c) AV+normalize,
    # O(c) out-projection. Emitted as S(c) / A(c-1) / O(c-2) so every
    # cross-engine dependency has ~a full chunk of slack to absorb the
    # ~150ns semaphore handoff latency of the hardware.
    otrs, egss, edss = {}, {}, {}

    def chunk_info(c):
        glob_in_chunk = [g for g in GLOB if g // 4 == c]
        lo = 128 if glob_in_chunk else 0
        qts = [4 * c + i for i in range(4) if (4 * c + i) not in GLOB]
        return glob_in_chunk, lo, qts

    def do_scores(c):
        _, _, qts = chunk_info(c)
        nq = len(qts)
        for h in (0, 1):
            hs = slice(h * 64, (h + 1) * 64)
            for g in GLOB:
                psg = ps_med.tile([P, 512], f32, tag="psmed")
                nc.tensor.matmul(
                    psg[:], kT[hs, g * P:(g + 1) * P],
                    qT[hs, c * 512:(c + 1) * 512],
                    start=True, stop=True,
                )
                eg = expp.tile([P, 512], bf16, tag="exp", bufs=12)
                nc.scalar.activation(eg[:], psg[:], Act.Exp, scale=SCALE)
                egss[c, g, h] = eg
            psd = ps_med.tile([P, 512], f32, tag="psmed")
            for idx, j in enumerate(qts):
                nc.tensor.matmul(
                    psd[:, idx * P:(idx + 1) * P],
                    kT[hs, j * P:(j + 1) * P], qT[hs, j * P:(j + 1) * P],
                    start=True, stop=True, skip_group_check=True,
                )
            ed = expp.tile([P, 512], bf16, tag="exp", bufs=12)
            nc.scalar.activation(
                ed[:, 0:nq * P], psd[:, 0:nq * P], Act.Exp, scale=SCALE
            )
            edss[c, h] = ed

    def do_av(c):
        glob_in_chunk, lo, qts = chunk_info(c)
        otr = smallp.tile([P, 512], bf16, tag="otr", name=f"otr{c}")
        otrs[c] = otr
        if glob_in_chunk:
            g = glob_in_chunk[0]
            gq_col = 0 if g == 0 else 128
            nc.vector.tensor_copy(otr[:, 0:128], gout[:, gq_col:gq_col + 128])
        pcs = {}
        for h in (0, 1):
            pc = ps_av.tile([65, 512], f32, tag="psav")
            pcs[h] = pc
            nc.tensor.matmul(pc[:, lo:512], vslice(GLOB[0], h),
                             egss[c, GLOB[0], h][:, lo:512],
                             start=True, stop=False)
            nc.tensor.matmul(pc[:, lo:512], vslice(GLOB[1], h),
                             egss[c, GLOB[1], h][:, lo:512],
                             start=False, stop=True)
            for idx, j in enumerate(qts):
                off = (j - 4 * c) * P
                nc.tensor.matmul(pc[:, off:off + P], vslice(j, h),
                                 edss[c, h][:, idx * P:(idx + 1) * P],
                                 start=False, stop=True,
                                 skip_group_check=True)  # sub-region accumulate
        # both l-extracts+broadcasts first, then both divides: DVE never
        # sits head-of-line waiting for a Pool broadcast round-trip.
        bsbs = {h: norm_prep(pcs[h], lo, 512) for h in (0, 1)}
        for h in (0, 1):
            norm_mult(pcs[h], bsbs[h], lo, 512, otr[h * 64:(h + 1) * 64, lo:512])

    def do_outproj(c):
        otr = otrs[c]
        for t in range(4):
            j = 4 * c + t
            osb = streamp.tile([P, D], bf16, tag="osb")
            for half in (0, 1):
                pso = ps_big.tile([P, 512], f32, tag="psbig")
                nc.tensor.matmul(
                    pso[:], otr[:, t * P:(t + 1) * P],
                    wo_r[:, half * 512:(half + 1) * 512],
                    start=True, stop=True,
                )
                if half == 0:
                    nc.vector.tensor_copy(osb[:, 0:512], pso[:])
                else:
                    nc.scalar.activation(osb[:, 512:1024], pso[:], Act.Identity)
            nc.gpsimd.dma_start(out[j * P:(j + 1) * P, :], osb[:])

    if PHASES == "qkv":
        do_quad(1)
        do_quad(2)
        return
    if PHASES == "attn1":
        do_quad(1); do_quad(2)
        do_scores(1)
        do_av(1)
        do_outproj(1)
        return
    if PHASES == "attn4":
        do_quad(1); do_quad(2)
        do_scores(1)
        do_scores(7); do_av(1)
        do_scores(2); do_av(7); do_outproj(1)
        do_scores(3); do_av(2); do_outproj(7)
        do_av(3); do_outproj(2)
        do_outproj(3)
        return
    # wavefront: S one chunk ahead of A, two ahead of O; quads 1 and 2 are
    # interleaved into the early slots (chunks 1 and 7 only need quads 0+3,
    # emitted above) so QKV matmuls fill the attention phase's wait gaps.
    # The global block (do_global) sits before the gout-dependent chunks 0/6.
    do_scores(1)
    do_quad(1)
    do_scores(7); do_av(1)
    do_quad(2)
    do_scores(2); do_av(7); do_outproj(1)
    do_scores(3); do_av(2); do_outproj(7)
    do_scores(4); do_av(3); do_outproj(2)
    do_scores(5); do_av(4); do_outproj(3)
    do_global(); do_outproj(4)
    do_av(5); do_outproj(5)
    do_scores(0); do_scores(6)
    do_av(0); do_outproj(0)
    do_av(6); do_outproj(6)


def _get_nc(reps=1):
    key = ("nc", reps)
    if key not in _CACHE:
        _CACHE[key] = _build_nc(reps)
    return _CACHE[key]


def _bf16(a):
    import ml_dtypes

    return np.asarray(a, dtype=np.float32).astype(ml_dtypes.bfloat16)


def _prep_inputs(x, w_qkv, b_qkv):
    x2 = np.asarray(x, dtype=np.float32).reshape(L, D)
    xT = _bf16(np.ascontiguousarray(x2.T))
    w_qkv = np.asarray(w_qkv, dtype=np.float32)
    b_qkv = np.asarray(b_qkv, dtype=np.float32)

    def tile_w(w_slice):
        wt = w_slice.T
        return _bf16(np.ascontiguousarray(
            wt.reshape(8, P, P).transpose(1, 0, 2).reshape(P, D)
        ))

    maps = []
    for c in range(8):
        a = 2 * c * HD
        b = a + 2 * HD
        maps.append({
            "xT": xT,
            "wq": tile_w(w_qkv[a:b, :]),
            "wk": tile_w(w_qkv[D + a:D + b, :]),
            "wv": tile_w(w_qkv[2 * D + a:2 * D + b, :]),
            "bq": np.ascontiguousarray(b_qkv[a:b].reshape(P, 1)),
            "bk": np.ascontiguousarray(b_qkv[D + a:D + b].reshape(P, 1)),
        })
    return maps


def kernel(x, w_qkv, b_qkv, w_out, b_out):
    from concourse.bass_utils import run_bass_kernel_spmd

    x = np.asarray(x, dtype=np.float32)
    w_qkv = np.asarray(w_qkv, dtype=np.float32)
    b_qkv = np.asarray(b_qkv, dtype=np.float32)
    w_out = np.asarray(w_out, dtype=np.float32)
    b_out = np.asarray(b_out, dtype=np.float32)

    nc = _get_nc()
    maps = _prep_inputs(x, w_qkv, b_qkv)
    for c in range(8):
        a = 2 * c * HD
        b = a + 2 * HD
        maps[c]["wo"] = _bf16(np.ascontiguousarray(w_out[:, a:b].T))

    res = run_bass_kernel_spmd(nc, maps, core_ids=list(range(8)))

    total = res.results[0]["out"].astype(np.float32)
    for c in range(1, 8):
        total += res.results[c]["out"].astype(np.float32)
    const_row = b_qkv[2 * D:3 * D] @ w_out.T + b_out
    total += const_row[None, :]
    return total.reshape(x.shape).astype(np.float32)


# revision 44
# speedup vs baseline: 1.5812x; 1.1028x over previous
"""Block-sparse attention Trainium2 kernel (v3, bf16 transposed-AV).

Reference: nn.MultiheadAttention-style block-sparse attention, B=1, L=4096,
D=1024, H=16, head_dim=64, block=128, global blocks {0, 24}.

Sharding: head-parallel across 8 cores (2 heads/core); host sums the 8
partial out-projections. The whole dataflow is bf16 (inputs pre-cast on
host): matmul operands bf16 into f32 PSUM, exps emit bf16, the partial
output is written bf16 and summed in f32 on host. Attention-value products
are computed in transposed form (outT = v_aug.T @ expT); softmax
denominators ride along as an extra ones-column of the augmented V;
normalization is a reciprocal + PE outer-product broadcast + one
elementwise multiply per 512-wide chunk.
"""

import sys

sys.path.insert(0, "/opt/trn_rl_repo")
import numpy as np

D = 1024
L = 4096
H = 16
HD = 64
NB = 32
GLOB = (0, 24)
P = 128
SCALE = 1.0 / 8.0

PHASES = "full"  # dev knob: "qkv" / "attn1" time sub-phases via mb_phase.py

_CACHE = {}


def _build_nc(reps=1):
    import contextlib

    import concourse.mybir as mybir
    import concourse.tile as tile
    from concourse import bacc
    from concourse.masks import make_identity

    f32 = mybir.dt.float32
    f32r = mybir.dt.float32r
    bf16 = mybir.dt.bfloat16
    Act = mybir.ActivationFunctionType
    AluMult = mybir.AluOpType.mult

    nc = bacc.Bacc("TRN2", target_bir_lowering=False, debug=False, num_devices=8)
    xT = nc.dram_tensor("xT", [D, L], bf16, kind="ExternalInput")
    wq = nc.dram_tensor("wq", [P, D], bf16, kind="ExternalInput")
    wk = nc.dram_tensor("wk", [P, D], bf16, kind="ExternalInput")
    wv = nc.dram_tensor("wv", [P, D], bf16, kind="ExternalInput")
    wo = nc.dram_tensor("wo", [P, D], bf16, kind="ExternalInput")
    bq = nc.dram_tensor("bq", [P, 1], f32, kind="ExternalInput")
    bk = nc.dram_tensor("bk", [P, 1], f32, kind="ExternalInput")
    out = nc.dram_tensor("out", [L, D], bf16, kind="ExternalOutput")

    with tile.TileContext(nc) as tc:
        with (
            tc.tile_pool(name="const", bufs=1) as constp,
            tc.tile_pool(name="stream", bufs=3) as streamp,
            tc.tile_pool(name="expb", bufs=6) as expp,
            tc.tile_pool(name="small", bufs=4) as smallp,
            tc.tile_pool(name="ps_big", bufs=2, space="PSUM") as ps_big,
            tc.tile_pool(name="ps_med", bufs=3, space="PSUM") as ps_med,
            tc.tile_pool(name="ps_av", bufs=3, space="PSUM") as ps_av,
        ):
            # ---------- constants / persistent buffers
            ident = constp.tile([P, P], bf16, tag="ident")
            make_identity(nc, ident[:])
            ones_col = constp.tile([P, 1], bf16, tag="ones")
            nc.vector.memset(ones_col[:], 1.0)

            wq_r = constp.tile([P, D], bf16, tag="wq_r")
            wk_r = constp.tile([P, D], bf16, tag="wk_r")
            wv_r = constp.tile([P, D], bf16, tag="wv_r")
            wo_r = constp.tile([P, D], bf16, tag="wo_r")
            for dram, tr in ((wq, wq_r), (wk, wk_r), (wv, wv_r), (wo, wo_r)):
                nc.sync.dma_start(tr[:], dram[:])
            bq_t = constp.tile([P, 1], f32, tag="bq")
            bk_t = constp.tile([P, 1], f32, tag="bk")
            nc.sync.dma_start(bq_t[:], bq[:])
            nc.sync.dma_start(bk_t[:], bk[:])

            qT = constp.tile([P, L], bf16, tag="qT")
            kT = constp.tile([P, L], bf16, tag="kT")
            vTf = constp.tile([P, L], bf16, tag="vTf")
            # augmented-V blocks: per block 2 windows of 128 cols, one per
            # head: [ones | 63 zeros | v.T (64)] — so each head's AV psum gets
            # l in row 0 (partition 0, where the custom DVE/gpsimd uops need
            # it) and data in the 64-aligned rows 64:128. Pad columns cost
            # nothing on PE (matmul cost is moving-dim rows only).
            vn = constp.tile([P, NB * 256], bf16, tag="vn")
            nc.vector.memset(vn[:], 0.0)
            qg = constp.tile([P, 256], bf16, tag="qg")
            gout = constp.tile([P, 256], bf16, tag="gout")
            for _b in range(NB):
                nc.vector.tensor_copy(vn[:, _b * 256:_b * 256 + 1], ones_col[:])
                nc.vector.tensor_copy(vn[:, _b * 256 + 128:_b * 256 + 129], ones_col[:])

            loop_ctx = tc.For_i(0, reps, 1) if reps > 1 else contextlib.nullcontext()
            with loop_ctx:
                _body(nc, tc, mybir, Act, f32, f32r, bf16, AluMult, locals())

    nc.compile()
    return nc


def _body(nc, tc, mybir, Act, f32, f32r, bf16, AluMult, env):
    constp = env["constp"]; streamp = env["streamp"]; expp = env["expp"]; smallp = env["smallp"]
    ps_big = env["ps_big"]; ps_med = env["ps_med"]; ps_av = env["ps_av"]
    ident = env["ident"]; ones_col = env["ones_col"]
    wq_r = env["wq_r"]; wk_r = env["wk_r"]; wv_r = env["wv_r"]; wo_r = env["wo_r"]
    bq_t = env["bq_t"]; bk_t = env["bk_t"]
    qT = env["qT"]; kT = env["kT"]; vTf = env["vTf"]; vn = env["vn"]
    qg = env["qg"]; gout = env["gout"]
    xT = env["xT"]; out = env["out"]
    AluAdd = mybir.AluOpType.add

    # ---------- phase A: qkv projections + fused v-transpose, per quad
    def do_quad(quad):
        xrs = []
        for kt in range(8):
            xraw = streamp.tile([P, 1024], bf16, tag="xraw", bufs=16)
            nc.sync.dma_start(
                xraw[:], xT[kt * P:(kt + 1) * P, quad * 1024:(quad + 1) * 1024]
            )
            xrs.append(xraw)
        for sub in range(2):
            n = quad * 2 + sub
            sl = slice(n * 512, (n + 1) * 512)
            for wt, dest, bias in (
                (wq_r, qT, bq_t),
                (wk_r, kT, bk_t),
                (wv_r, vTf, None),
            ):
                pp = ps_big.tile([P, 512], f32, tag="psbig")
                for kt in range(8):
                    nc.tensor.matmul(
                        pp[:], wt[:, kt * P:(kt + 1) * P],
                        xrs[kt][:, sub * 512:(sub + 1) * 512],
                        start=kt == 0, stop=kt == 7,
                    )
                if bias is not None:
                    nc.scalar.activation(dest[:, sl], pp[:], Act.Identity, bias=bias[:])
                else:
                    nc.vector.tensor_copy(dest[:, sl], pp[:])
        for b in range(8 * quad, 8 * quad + 8):
            pst = ps_av.tile([P, P], bf16, tag="psav", name=f"pst{b}")
            nc.tensor.transpose(pst[:], vTf[:, b * P:(b + 1) * P], ident[:])
            base = b * 256
            nc.vector.tensor_copy(vn[:, base + 64:base + 128], pst[:, 0:64])
            nc.vector.tensor_copy(vn[:, base + 192:base + 256], pst[:, 64:128])

    do_quad(0)
    do_quad(3)
    # stage global-q columns (available after quads 0 and 3)
    nc.vector.tensor_copy(qg[:, 0:128], qT[:, 0:128])
    nc.vector.tensor_copy(qg[:, 128:256], qT[:, GLOB[1] * P:(GLOB[1] + 1) * P])

    def vslice(blk, h):
        return vn[:, blk * 256 + h * 128: blk * 256 + (h + 1) * 128]

    def norm_prep(src, lo, hi, l_in_sbuf=False):
        # src [128, W+]: row 0 = l (partition 0 — where the custom DVE/gpsimd
        # uops need it on HW), rows 64:128 = unnormalized outT.
        # Returns [64, W] broadcast of 1/l. reciprocal_approx_fast: the exact
        # DVE reciprocal costs ~6ns per free-element on HW (~2.9us per row),
        # the approx one ~0.7ns; softmax denominators don't need the bits.
        W = hi - lo
        linv = smallp.tile([1, 512], f32, tag="linv")
        with nc.allow_low_precision(reason="softmax denom tolerates approx"):
            nc.vector.reciprocal_approx_fast(linv[0:1, 0:W], src[0:1, lo:hi])
        bsb = smallp.tile([64, 512], f32, tag="bsb")
        nc.gpsimd.partition_broadcast(bsb[0:64, 0:W], linv[0:1, 0:W])
        return bsb

    def norm_mult(src, bsb, lo, hi, dest):
        W = hi - lo
        nc.vector.tensor_tensor(
            dest, src[64:128, lo:hi], bsb[0:64, 0:W], AluMult
        )

    def normalize_emit(src, h, lo, hi, dest, l_in_sbuf=False):
        norm_mult(src, norm_prep(src, lo, hi, l_in_sbuf), lo, hi, dest)

    # ---------- global qtiles (0 and 24): attend to all 32 blocks.
    # pg comes from the ps_av pool: during do_global no chunk AVs run, so the
    # pool's rotation has slack for one long-lived accumulator per head.
    def do_global():
      for h in (0, 1):
        hs = slice(h * 64, (h + 1) * 64)
        pg = ps_av.tile([P, 256], f32, tag="psav", name=f"pg{h}")
        for kb2 in range(NB // 2):
            psg = ps_med.tile([P, 512], f32, tag="psmed")
            for half in (0, 1):
                kb = 2 * kb2 + half
                nc.tensor.matmul(
                    psg[:, half * 256:(half + 1) * 256],
                    kT[hs, kb * P:(kb + 1) * P], qg[hs, :],
                    start=True, stop=True,
                )
            eg = expp.tile([P, 512], bf16, tag="gexp")
            nc.scalar.activation(eg[:], psg[:], Act.Exp, scale=SCALE)
            for half in (0, 1):
                kb = 2 * kb2 + half
                nc.tensor.matmul(
                    pg[:], vslice(kb, h), eg[:, half * 256:(half + 1) * 256],
                    start=kb == 0, stop=kb == NB - 1,
                )
        normalize_emit(pg, h, 0, 256, gout[h * 64:(h + 1) * 64, :])

    # ---------- chunk pipeline: S(c) scores+exps, A(c) AV+normalize,
    # O(c) out-projection. Emitted as S(c) / A(c-1) / O(c-2) so every
    # cross-engine dependency has ~a full chunk of slack to absorb the
    # ~150ns semaphore handoff latency of the hardware.
    otrs, egss, edss = {}, {}, {}

    def chunk_info(c):
        glob_in_chunk = [g for g in GLOB if g // 4 == c]
        lo = 128 if glob_in_chunk else 0
        qts = [4 * c + i for i in range(4) if (4 * c + i) not in GLOB]
        return glob_in_chunk, lo, qts

    def do_scores(c):
        _, _, qts = chunk_info(c)
        nq = len(qts)
        for h in (0, 1):
            hs = slice(h * 64, (h + 1) * 64)
            for g in GLOB:
                psg = ps_med.tile([P, 512], f32, tag="psmed")
                nc.tensor.matmul(
                    psg[:], kT[hs, g * P:(g + 1) * P],
                    qT[hs, c * 512:(c + 1) * 512],
                    start=True, stop=True,
                )
                eg = expp.tile([P, 512], bf16, tag="exp", bufs=12)
                nc.scalar.activation(eg[:], psg[:], Act.Exp, scale=SCALE)
                egss[c, g, h] = eg
            psd = ps_med.tile([P, 512], f32, tag="psmed")
            for idx, j in enumerate(qts):
                nc.tensor.matmul(
                    psd[:, idx * P:(idx + 1) * P],
                    kT[hs, j * P:(j + 1) * P], qT[hs, j * P:(j + 1) * P],
                    start=True, stop=True, skip_group_check=True,
                )
            ed = expp.tile([P, 512], bf16, tag="exp", bufs=12)
            nc.scalar.activation(
                ed[:, 0:nq * P], psd[:, 0:nq * P], Act.Exp, scale=SCALE
            )
            edss[c, h] = ed

    def do_av(c):
        glob_in_chunk, lo, qts = chunk_info(c)
        otr = smallp.tile([P, 512], bf16, tag="otr", name=f"otr{c}")
        otrs[c] = otr
        if glob_in_chunk:
            g = glob_in_chunk[0]
            gq_col = 0 if g == 0 else 128
            nc.vector.tensor_copy(otr[:, 0:128], gout[:, gq_col:gq_col + 128])
        pcs = {}
        for h in (0, 1):
            pc = ps_av.tile([P, 512], f32, tag="psav")
            pcs[h] = pc
            nc.tensor.matmul(pc[:, lo:512], vslice(GLOB[0], h),
                             egss[c, GLOB[0], h][:, lo:512],
                             start=True, stop=False)
            nc.tensor.matmul(pc[:, lo:512], vslice(GLOB[1], h),
                             egss[c, GLOB[1], h][:, lo:512],
                             start=False, stop=True)
            for idx, j in enumerate(qts):
                off = (j - 4 * c) * P
                nc.tensor.matmul(pc[:, off:off + P], vslice(j, h),
                                 edss[c, h][:, idx * P:(idx + 1) * P],
                                 start=False, stop=True,
                                 skip_group_check=True)  # sub-region accumulate
        # both l-extracts+broadcasts first, then both divides: DVE never
        # sits head-of-line waiting for a Pool broadcast round-trip.
        bsbs = {h: norm_prep(pcs[h], lo, 512) for h in (0, 1)}
        for h in (0, 1):
            norm_mult(pcs[h], bsbs[h], lo, 512, otr[h * 64:(h + 1) * 64, lo:512])

    def do_outproj(c):
        otr = otrs[c]
        for t in range(4):
            j = 4 * c + t
            osb = streamp.tile([P, D], bf16, tag="osb")
            for half in (0, 1):
                pso = ps_big.tile([P, 512], f32, tag="psbig")
                nc.tensor.matmul(
                    pso[:], otr[:, t * P:(t + 1) * P],
                    wo_r[:, half * 512:(half + 1) * 512],
                    start=True, stop=True,
                )
                if half == 0:
                    nc.vector.tensor_copy(osb[:, 0:512], pso[:])
                else:
                    nc.scalar.activation(osb[:, 512:1024], pso[:], Act.Identity)
            nc.gpsimd.dma_start(out[j * P:(j + 1) * P, :], osb[:])

    if PHASES == "qkv":
        do_quad(1)
        do_quad(2)
        return
    if PHASES == "attn1":
        do_quad(1); do_quad(2)
        do_scores(1)
        do_av(1)
        do_outproj(1)
        return
    if PHASES == "attn4":
        do_quad(1); do_quad(2)
        do_scores(1)
        do_scores(7); do_av(1)
        do_scores(2); do_av(7); do_outproj(1)
        do_scores(3); do_av(2); do_outproj(7)
        do_av(3); do_outproj(2)
        do_outproj(3)
        return
    # wavefront: S one chunk ahead of A, two ahead of O; quads 1 and 2 are
    # interleaved into the early slots (chunks 1 and 7 only need quads 0+3,
    # emitted above) so QKV matmuls fill the attention phase's wait gaps.
    # The global block (do_global) sits before the gout-dependent chunks 0/6.
    do_scores(1)
    do_quad(1)
    do_scores(7); do_av(1)
    do_quad(2)
    do_scores(2); do_av(7); do_outproj(1)
    do_scores(3); do_av(2); do_outproj(7)
    do_scores(4); do_av(3); do_outproj(2)
    do_scores(5); do_av(4); do_outproj(3)
    do_global(); do_outproj(4)
    do_av(5); do_outproj(5)
    do_scores(0); do_scores(6)
    do_av(0); do_outproj(0)
    do_av(6); do_outproj(6)


def _get_nc(reps=1):
    key = ("nc", reps)
    if key not in _CACHE:
        _CACHE[key] = _build_nc(reps)
    return _CACHE[key]


def _bf16(a):
    import ml_dtypes

    return np.asarray(a, dtype=np.float32).astype(ml_dtypes.bfloat16)


def _prep_inputs(x, w_qkv, b_qkv):
    x2 = np.asarray(x, dtype=np.float32).reshape(L, D)
    xT = _bf16(np.ascontiguousarray(x2.T))
    w_qkv = np.asarray(w_qkv, dtype=np.float32)
    b_qkv = np.asarray(b_qkv, dtype=np.float32)

    def tile_w(w_slice):
        wt = w_slice.T
        return _bf16(np.ascontiguousarray(
            wt.reshape(8, P, P).transpose(1, 0, 2).reshape(P, D)
        ))

    maps = []
    for c in range(8):
        a = 2 * c * HD
        b = a + 2 * HD
        maps.append({
            "xT": xT,
            "wq": tile_w(w_qkv[a:b, :]),
            "wk": tile_w(w_qkv[D + a:D + b, :]),
            "wv": tile_w(w_qkv[2 * D + a:2 * D + b, :]),
            "bq": np.ascontiguousarray(b_qkv[a:b].reshape(P, 1)),
            "bk": np.ascontiguousarray(b_qkv[D + a:D + b].reshape(P, 1)),
        })
    return maps


def kernel(x, w_qkv, b_qkv, w_out, b_out):
    from concourse.bass_utils import run_bass_kernel_spmd

    x = np.asarray(x, dtype=np.float32)
    w_qkv = np.asarray(w_qkv, dtype=np.float32)
    b_qkv = np.asarray(b_qkv, dtype=np.float32)
    w_out = np.asarray(w_out, dtype=np.float32)
    b_out = np.asarray(b_out, dtype=np.float32)

    nc = _get_nc()
    maps = _prep_inputs(x, w_qkv, b_qkv)
    for c in range(8):
        a = 2 * c * HD
        b = a + 2 * HD
        maps[c]["wo"] = _bf16(np.ascontiguousarray(w_out[:, a:b].T))

    res = run_bass_kernel_spmd(nc, maps, core_ids=list(range(8)))

    total = res.results[0]["out"].astype(np.float32)
    for c in range(1, 8):
        total += res.results[c]["out"].astype(np.float32)
    const_row = b_qkv[2 * D:3 * D] @ w_out.T + b_out
    total += const_row[None, :]
    return total.reshape(x.shape).astype(np.float32)
